# revision 2
# baseline (speedup 1.0000x reference)
"""ConvNet (conv1d + bias + relu + max/avg pool + MLP head) on 8 TRN2 cores.

Strategy: pure data-parallel over batch (32 batches/core).

Per core:
  - conv1d is an im2col matmul with contraction K = C_IN*GLEN + 1 = 97; the
    extra "ones" row of the rhs times a wRect row in lhsT adds the conv bias
    inside the matmul, so PSUM holds w = conv + bias directly.
  - per (batch, channel-half) unit ([128 ch, 2025 pos] in 4 PSUM banks):
      ScalarE: relu(w) -> SBUF scratch, fused accum_out = sum-pool (exact).
      VectorE: fused tensor_scalar(op1=max) max-reduce over relu'd scratch
               (fp32 SBUF single-src -> 2x_2p mode, 2 elem/cycle).
    A fraction of units run both passes on VectorE to balance ACT/DVE load.
  - pool stats land as [k, batch] tiles == transposed lhsT chunks for the
    MLP head; mean's 1/2025, DROPPROB, and all bias folds are precomputed
    on the host into the weight tensors.
"""

import numpy as np

B, C_IN, L = 256, 4, 2048
HNODE, GLEN = 256, 24
OUTL = L - GLEN + 1  # 2025
NCORES = 8
BSH = B // NCORES  # 32
KAUG = C_IN * GLEN + 1  # 97
SCR_W = OUTL + 1  # 2026 (even free dim for DVE 2x_2p mode)
DROPPROB = 0.5

_cache: dict = {}


def _build_program():
    import concourse.bacc as bacc
    import concourse.mybir as mybir
    import concourse.tile as tile
    from concourse.ap import AP

    f32 = mybir.dt.float32
    AOP = mybir.AluOpType
    AF = mybir.ActivationFunctionType

    nc = bacc.Bacc(
        "TRN2", target_bir_lowering=False, debug=False, num_devices=NCORES
    )
    x_d = nc.dram_tensor("x", [BSH, C_IN, L], f32, kind="ExternalInput")
    wconv_d = nc.dram_tensor("wconvT", [KAUG, HNODE], f32, kind="ExternalInput")
    whid_d = nc.dram_tensor("whid", [2 * HNODE, 2 * HNODE], f32, kind="ExternalInput")
    whb_d = nc.dram_tensor("whbias", [128, 4], f32, kind="ExternalInput")
    wneu_d = nc.dram_tensor("wneu", [128, 4], f32, kind="ExternalInput")
    wnb_d = nc.dram_tensor("wnbias", [1, 1], f32, kind="ExternalInput")
    out_d = nc.dram_tensor("out", [BSH, 1], f32, kind="ExternalOutput")

    with tile.TileContext(nc) as tc:
        with (
            tc.tile_pool(name="const", bufs=1) as constp,
            tc.tile_pool(name="xcol", bufs=3) as xcolp,
            tc.tile_pool(name="scratch", bufs=3) as scrp,
            tc.tile_pool(name="dummy", bufs=1) as dummyp,
            tc.tile_pool(name="stats", bufs=1) as statp,
        ):
            wconv_t = constp.tile([KAUG, HNODE], f32, tag="wconv")
            nc.sync.dma_start(wconv_t[:], wconv_d[:])
            whid_t = []
            for i in range(4):
                t = constp.tile([128, 512], f32, tag=f"whid{i}")
                nc.sync.dma_start(t[:], whid_d[128 * i : 128 * (i + 1), :])
                whid_t.append(t)
            whb_t = constp.tile([128, 4], f32, tag="whb")
            nc.sync.dma_start(whb_t[:], whb_d[:])
            wneu_t = constp.tile([128, 4], f32, tag="wneu")
            nc.sync.dma_start(wneu_t[:], wneu_d[:])
            wnb_t = constp.tile([1, 1], f32, tag="wnb")
            nc.sync.dma_start(wnb_t[:], wnb_d[:])

            # im2col buffers; row 96 is the constant "ones" row (bias trick).
            xts = [xcolp.tile([KAUG, OUTL], f32, tag="xc", name=f"xc{i}") for i in range(3)]
            for t in xts:
                nc.vector.memset(t[KAUG - 1 : KAUG, :], 1.0)
            # relu'd scratch; final pad column keeps the max-reduce width even.
            scrs = [scrp.tile([128, SCR_W], f32, tag="scr", name=f"scr{i}") for i in range(3)]
            for t in scrs:
                nc.vector.memset(t[:, OUTL:SCR_W], -1e30)
            dummy = dummyp.tile([128, SCR_W], f32, tag="dum")
            mx = [statp.tile([128, BSH], f32, tag=f"mx{h}", name=f"mx{h}") for h in range(2)]
            sm = [statp.tile([128, BSH], f32, tag=f"sm{h}", name=f"sm{h}") for h in range(2)]

            with tc.tile_pool(name="psum", bufs=2, space="PSUM") as psump:
                for b in range(BSH):
                    xt = xts[b % 3]
                    src = AP(
                        tensor=x_d,
                        offset=b * C_IN * L,
                        ap=[[L, C_IN], [1, GLEN], [1, OUTL]],
                    )
                    nc.sync.dma_start(xt[: KAUG - 1, :], src)
                    for h in range(2):
                        u = 2 * b + h
                        ps = psump.tile([128, 2048], f32, tag="conv")
                        for t4 in range(4):
                            n0 = 512 * t4
                            n1 = min(OUTL, n0 + 512)
                            nc.tensor.matmul(
                                ps[:, n0:n1],
                                wconv_t[:, 128 * h : 128 * (h + 1)],
                                xt[:, n0:n1],
                                start=True,
                                stop=True,
                            )
                        scr = scrs[u % 3]
                        if u % 6 == 5:
                            # DVE takes the whole unit (load balancing)
                            nc.vector.tensor_scalar(
                                out=scr[:, :OUTL],
                                in0=ps[:, :OUTL],
                                scalar1=0.0,
                                scalar2=None,
                                op0=AOP.max,
                                op1=AOP.add,
                                accum_out=sm[h][:, b : b + 1],
                            )
                        else:
                            nc.scalar.activation(
                                scr[:, :OUTL],
                                ps[:, :OUTL],
                                AF.Relu,
                                accum_out=sm[h][:, b : b + 1],
                            )
                        nc.vector.tensor_scalar(
                            out=dummy[:],
                            in0=scr[:],
                            scalar1=0.0,
                            scalar2=None,
                            op0=AOP.add,
                            op1=AOP.max,
                            accum_out=mx[h][:, b : b + 1],
                        )

            # MLP head. pool^T chunks along k: [max ch0-127, max ch128-255,
            # sum ch0-127, sum ch128-255] -- exactly mx[0], mx[1], sm[0], sm[1].
            with (
                tc.tile_pool(name="psum2", bufs=2, space="PSUM") as psump2,
                tc.tile_pool(name="head", bufs=1) as headp,
            ):
                poolk = [mx[0], mx[1], sm[0], sm[1]]
                hid_t = []
                for j in range(4):
                    psh = psump2.tile([128, BSH], f32, tag="hid")
                    for kc in range(4):
                        nc.tensor.matmul(
                            psh[:],
                            whid_t[kc][:, 128 * j : 128 * (j + 1)],
                            poolk[kc][:],
                            start=(kc == 0),
                            stop=(kc == 3),
                        )
                    ht = headp.tile([128, BSH], f32, tag=f"hid{j}")
                    nc.scalar.activation(
                        ht[:], psh[:], AF.Relu, bias=whb_t[:, j : j + 1]
                    )
                    hid_t.append(ht)
                pso = psump2.tile([1, BSH], f32, tag="out")
                for j in range(4):
                    nc.tensor.matmul(
                        pso[:],
                        wneu_t[:, j : j + 1],
                        hid_t[j][:],
                        start=(j == 0),
                        stop=(j == 3),
                    )
                out_sb = headp.tile([1, BSH], f32, tag="outsb")
                nc.scalar.activation(out_sb[:], pso[:], AF.Identity, bias=wnb_t[:])
                nc.sync.dma_start(out_d[:].rearrange("b one -> one b"), out_sb[:])

    nc.compile()
    return nc


def _get_program():
    if "nc" not in _cache:
        _cache["nc"] = _build_program()
    return _cache["nc"]


def _prep_weights(wConv, wRect, wHidden, wHiddenBias, wNeu, wNeuBias):
    wcT = np.zeros((KAUG, HNODE), np.float32)
    wcT[: KAUG - 1] = np.transpose(np.asarray(wConv, np.float32), (1, 2, 0)).reshape(
        C_IN * GLEN, HNODE
    )
    wcT[KAUG - 1] = np.asarray(wRect, np.float32)
    whid = np.array(wHidden, np.float32)
    whid[HNODE:, :] /= float(OUTL)  # fold avg-pool's 1/2025
    whb = np.ascontiguousarray(
        np.asarray(wHiddenBias, np.float32).reshape(4, 128).T
    )
    wneu = np.ascontiguousarray(
        (DROPPROB * np.asarray(wNeu, np.float32)[:, 0]).reshape(4, 128).T
    )
    wnb = np.asarray(wNeuBias, np.float32).reshape(1, 1)
    return wcT, whid, whb, wneu, wnb


def kernel(x, wConv, wRect, wHidden, wHiddenBias, wNeu, wNeuBias):
    from concourse import bass_utils

    nc = _get_program()
    wcT, whid, whb, wneu, wnb = _prep_weights(
        wConv, wRect, wHidden, wHiddenBias, wNeu, wNeuBias
    )
    x = np.ascontiguousarray(np.asarray(x, np.float32))
    in_maps = [
        {
            "x": np.ascontiguousarray(x[i * BSH : (i + 1) * BSH]),
            "wconvT": wcT,
            "whid": whid,
            "whbias": whb,
            "wneu": wneu,
            "wnbias": wnb,
        }
        for i in range(NCORES)
    ]
    res = bass_utils.run_bass_kernel_spmd(nc, in_maps, core_ids=list(range(NCORES)))
    return np.concatenate([res.results[i]["out"] for i in range(NCORES)], axis=0)


# revision 5
# speedup vs baseline: 1.5400x; 1.5400x over previous
"""ConvNet (conv1d + bias + relu + max/avg pool + MLP head) on 8 TRN2 cores.

Strategy: pure data-parallel over batch (32 batches/core).

Per core:
  - conv1d is an im2col matmul with contraction K = C_IN*GLEN + 1 = 97; the
    extra "ones" row of the rhs times a wRect row in lhsT adds the conv bias
    inside the matmul, so PSUM holds w = conv + bias directly.
  - per (batch, channel-half) unit ([128 ch, 2025 pos] in 4 PSUM banks):
      ScalarE: relu(w) -> SBUF scratch, fused accum_out = sum-pool (exact).
      VectorE: fused tensor_scalar(op1=max) max-reduce over relu'd scratch
               (fp32 SBUF single-src -> 2x_2p mode, 2 elem/cycle).
    A fraction of units run both passes on VectorE to balance ACT/DVE load.
  - pool stats land as [k, batch] tiles == transposed lhsT chunks for the
    MLP head; mean's 1/2025, DROPPROB, and all bias folds are precomputed
    on the host into the weight tensors.
"""

import numpy as np

B, C_IN, L = 256, 4, 2048
HNODE, GLEN = 256, 24
OUTL = L - GLEN + 1  # 2025
NCORES = 8
BSH = B // NCORES  # 32
KAUG = C_IN * GLEN + 1  # 97
SCR_W = OUTL + 1  # 2026 (even free dim for DVE 2x_2p mode)
DROPPROB = 0.5

_cache: dict = {}


def _build_program():
    import concourse.bacc as bacc
    import concourse.mybir as mybir
    import concourse.tile as tile
    from concourse.ap import AP

    f32 = mybir.dt.float32
    bf16 = mybir.dt.bfloat16
    AOP = mybir.AluOpType
    AF = mybir.ActivationFunctionType

    nc = bacc.Bacc(
        "TRN2", target_bir_lowering=False, debug=False, num_devices=NCORES
    )
    x_d = nc.dram_tensor("x", [BSH, C_IN, L], bf16, kind="ExternalInput")
    wconv_d = nc.dram_tensor("wconvT", [KAUG, HNODE], bf16, kind="ExternalInput")
    whid_d = nc.dram_tensor("whid", [2 * HNODE, 2 * HNODE], f32, kind="ExternalInput")
    whb_d = nc.dram_tensor("whbias", [128, 4], f32, kind="ExternalInput")
    wneu_d = nc.dram_tensor("wneu", [128, 4], f32, kind="ExternalInput")
    wnb_d = nc.dram_tensor("wnbias", [1, 1], f32, kind="ExternalInput")
    out_d = nc.dram_tensor("out", [BSH, 1], f32, kind="ExternalOutput")

    with tile.TileContext(nc) as tc:
        with (
            tc.tile_pool(name="const", bufs=1) as constp,
            tc.tile_pool(name="xcol", bufs=3) as xcolp,
            tc.tile_pool(name="scratch", bufs=3) as scrp,
            tc.tile_pool(name="dummy", bufs=1) as dummyp,
            tc.tile_pool(name="stats", bufs=1) as statp,
        ):
            wconv_t = constp.tile([KAUG, HNODE], bf16, tag="wconv")
            nc.sync.dma_start(wconv_t[:], wconv_d[:])
            whid_t = []
            for i in range(4):
                t = constp.tile([128, 512], f32, tag=f"whid{i}")
                nc.sync.dma_start(t[:], whid_d[128 * i : 128 * (i + 1), :])
                whid_t.append(t)
            whb_t = constp.tile([128, 4], f32, tag="whb")
            nc.sync.dma_start(whb_t[:], whb_d[:])
            wneu_t = constp.tile([128, 4], f32, tag="wneu")
            nc.sync.dma_start(wneu_t[:], wneu_d[:])
            wnb_t = constp.tile([1, 1], f32, tag="wnb")
            nc.sync.dma_start(wnb_t[:], wnb_d[:])

            # im2col buffers; row 96 is the constant "ones" row (bias trick).
            xts = [xcolp.tile([KAUG, OUTL], bf16, tag="xc", name=f"xc{i}") for i in range(3)]
            for t in xts:
                nc.vector.memset(t[KAUG - 1 : KAUG, :], 1.0)
            # relu'd scratch; final pad column keeps the max-reduce width even.
            scrs = [scrp.tile([128, SCR_W], f32, tag="scr", name=f"scr{i}") for i in range(3)]
            for t in scrs:
                nc.vector.memset(t[:, OUTL:SCR_W], -1e30)
            dummy = dummyp.tile([128, SCR_W], f32, tag="dum")
            mx = [statp.tile([128, BSH], f32, tag=f"mx{h}", name=f"mx{h}") for h in range(2)]
            sm = [statp.tile([128, BSH], f32, tag=f"sm{h}", name=f"sm{h}") for h in range(2)]

            with tc.tile_pool(name="psum", bufs=2, space="PSUM") as psump:
                for b in range(BSH):
                    xt = xts[b % 3]
                    src = AP(
                        tensor=x_d,
                        offset=b * C_IN * L,
                        ap=[[L, C_IN], [1, GLEN], [1, OUTL]],
                    )
                    # SWDGE: sprays descriptors across the full SDMA engine
                    # set (HWDGE's ring only used 4 engines -> 100 GB/s cap).
                    nc.gpsimd.dma_start(xt[: KAUG - 1, :], src)
                    for h in range(2):
                        u = 2 * b + h
                        ps = psump.tile([128, 2048], f32, tag="conv")
                        for t4 in range(4):
                            n0 = 512 * t4
                            n1 = min(OUTL, n0 + 512)
                            nc.tensor.matmul(
                                ps[:, n0:n1],
                                wconv_t[:, 128 * h : 128 * (h + 1)],
                                xt[:, n0:n1],
                                start=True,
                                stop=True,
                            )
                        scr = scrs[u % 3]
                        nc.scalar.activation(
                            scr[:, :OUTL],
                            ps[:, :OUTL],
                            AF.Relu,
                            accum_out=sm[h][:, b : b + 1],
                        )
                        nc.vector.tensor_scalar(
                            out=dummy[:],
                            in0=scr[:],
                            scalar1=0.0,
                            scalar2=None,
                            op0=AOP.add,
                            op1=AOP.max,
                            accum_out=mx[h][:, b : b + 1],
                        )

            # MLP head. pool^T chunks along k: [max ch0-127, max ch128-255,
            # sum ch0-127, sum ch128-255] -- exactly mx[0], mx[1], sm[0], sm[1].
            with (
                tc.tile_pool(name="psum2", bufs=2, space="PSUM") as psump2,
                tc.tile_pool(name="head", bufs=1) as headp,
            ):
                poolk = [mx[0], mx[1], sm[0], sm[1]]
                hid_t = []
                for j in range(4):
                    psh = psump2.tile([128, BSH], f32, tag="hid")
                    for kc in range(4):
                        nc.tensor.matmul(
                            psh[:],
                            whid_t[kc][:, 128 * j : 128 * (j + 1)],
                            poolk[kc][:],
                            start=(kc == 0),
                            stop=(kc == 3),
                        )
                    ht = headp.tile([128, BSH], f32, tag=f"hid{j}")
                    nc.scalar.activation(
                        ht[:], psh[:], AF.Relu, bias=whb_t[:, j : j + 1]
                    )
                    hid_t.append(ht)
                pso = psump2.tile([1, BSH], f32, tag="out")
                for j in range(4):
                    nc.tensor.matmul(
                        pso[:],
                        wneu_t[:, j : j + 1],
                        hid_t[j][:],
                        start=(j == 0),
                        stop=(j == 3),
                    )
                out_sb = headp.tile([1, BSH], f32, tag="outsb")
                nc.scalar.activation(out_sb[:], pso[:], AF.Identity, bias=wnb_t[:])
                nc.sync.dma_start(out_d[:].rearrange("b one -> one b"), out_sb[:])

    nc.compile()
    return nc


def _get_program():
    if "nc" not in _cache:
        _cache["nc"] = _build_program()
    return _cache["nc"]


def _prep_weights(wConv, wRect, wHidden, wHiddenBias, wNeu, wNeuBias):
    import ml_dtypes

    wcT = np.zeros((KAUG, HNODE), np.float32)
    wcT[: KAUG - 1] = np.transpose(np.asarray(wConv, np.float32), (1, 2, 0)).reshape(
        C_IN * GLEN, HNODE
    )
    wcT[KAUG - 1] = np.asarray(wRect, np.float32)
    wcT = wcT.astype(ml_dtypes.bfloat16)
    whid = np.array(wHidden, np.float32)
    whid[HNODE:, :] /= float(OUTL)  # fold avg-pool's 1/2025
    whb = np.ascontiguousarray(
        np.asarray(wHiddenBias, np.float32).reshape(4, 128).T
    )
    wneu = np.ascontiguousarray(
        (DROPPROB * np.asarray(wNeu, np.float32)[:, 0]).reshape(4, 128).T
    )
    wnb = np.asarray(wNeuBias, np.float32).reshape(1, 1)
    return wcT, whid, whb, wneu, wnb


def kernel(x, wConv, wRect, wHidden, wHiddenBias, wNeu, wNeuBias):
    from concourse import bass_utils

    nc = _get_program()
    wcT, whid, whb, wneu, wnb = _prep_weights(
        wConv, wRect, wHidden, wHiddenBias, wNeu, wNeuBias
    )
    import ml_dtypes

    x = np.ascontiguousarray(np.asarray(x, np.float32).astype(ml_dtypes.bfloat16))
    in_maps = [
        {
            "x": np.ascontiguousarray(x[i * BSH : (i + 1) * BSH]),
            "wconvT": wcT,
            "whid": whid,
            "whbias": whb,
            "wneu": wneu,
            "wnbias": wnb,
        }
        for i in range(NCORES)
    ]
    res = bass_utils.run_bass_kernel_spmd(nc, in_maps, core_ids=list(range(NCORES)))
    return np.concatenate([res.results[i]["out"] for i in range(NCORES)], axis=0)
